# revision 24
# baseline (speedup 1.0000x reference)
"""Trainium2 Bass kernel for nn_Attention_1322849927460.

Dense transformer block: LN -> qkv -> attention (+ spatial-bias MLP on
attention-weighted coordinate deltas) -> out proj -> gelu -> residual.

Sharding: 8 cores = (2 batches) x (4 sequence quarters). Each core holds
all 8 heads for its 512 query rows and the full 2048-token K/V of its
batch, so no collectives are needed. A host-side roll of the token axis
puts each core's query rows first, letting all cores run an identical
SPMD program (attention is invariant to key-order permutation).

Algebraic structure:
  * delta_full[b,h,i,:] = (attn @ xyz)[b,h,i,:] - xyz[b,i,:] since softmax
    rows sum to one -> the (m,m,3) delta tensor is never formed.
  * softmax denominators come free from augmented V' columns [xyz/32, 1/32];
    one reciprocal + partition-broadcast normalizes the accumulators.  The
    1/32 ones-column also lands the normalized attention output at 32x
    true scale, lifting the fp8 outfin tensor out of e4m3 denormals free.
  * ln_g and the 1/sqrt(dh) q-scale fold into the qkv weights on host.

fp8 strategy (vs the bf16 baseline):
  * all projection matmuls run in fp8e4 (same column cost as bf16, but
    half the weight DMA); AV runs as fp8 DoubleRow over adjacent j-tile
    pairs (2 k-tiles per instruction, halving AV instruction count), with
    the xyz/ones columns as a second tiny DoubleRow into a base-0 [4, .]
    accumulator (DoubleRow outputs must start at partition 0, and the
    stationary k-tile stride must be a power of two - walrus crashes
    otherwise, hence split vv/vx tiles).  Spatial-MLP h2 is DoubleRow
    over kc pairs.  QK and MLP h1 stay bf16 (PSUM-output-bound).
  * weights quantized with power-of-2 host scales (wqkv x64, spw2 x32,
    wout x64), descaled for free inside evacuation ops / the final ACT.
  * exp alternates per j-tile: even tiles exact on the scalar engine,
    odd tiles on the DVE via Schraudolph into e4m3 bits.  Both write an
    int16-spaced e buffer (DVE needs a 2-byte output dtype for its 2x
    mode; i16 = round(x*8/ln2 + 55.66) has the e4m3 bit pattern in its
    low byte) and AV reads a stride-2 fp8 view.
"""

import os
import sys

for _p in ("/opt/trn_rl_repo",):
    if _p not in sys.path and os.path.isdir(_p):
        sys.path.insert(0, _p)

import ml_dtypes
import numpy as np

import concourse.bass as bass
import concourse.bacc as bacc
import concourse.tile as tile
from concourse import mybir
from concourse.bass_utils import run_bass_kernel_spmd
from concourse.masks import make_identity

F32 = mybir.dt.float32
BF16 = mybir.dt.bfloat16
F8 = mybir.dt.float8e4
I16 = mybir.dt.int16
AF = mybir.ActivationFunctionType
OP = mybir.AluOpType
DR = mybir.MatmulPerfMode.DoubleRow
BF = ml_dtypes.bfloat16
E4 = ml_dtypes.float8_e4m3

DIM = 256
H = 8
DH = 64
INNER = H * DH  # 512
M = 2048  # tokens per batch
TQ = 512  # query tokens per core
NT = M // 128  # 16 token tiles
N_CORES = 8
LN_EPS = 1e-5

# host-side fp8 weight scales (powers of two; descaled on-chip for free)
WQ_S = 64.0      # wqkv scale; descale 2^-6 in the q/k/v evacuations
W2_S = 32.0      # spw2 scale == the outfin x32 target scale
WO_S = 64.0      # wout scale; total descale 2^-11 in the final gelu
OF_S = 32.0      # outfin scale, produced by the 1/32 ones/xyz columns
WQ_INV = 1.0 / WQ_S
YT_INV = 1.0 / (OF_S * WO_S)

# Schraudolph fast exp in e4m3 bit domain:
#   e4m3(x) bits = round(x * 2^3/ln2 + (7*2^3 - 0.34)) viewed as int8.
EXP_A8 = 8.0 / float(np.log(2.0))
EXP_B8 = 56.0 - 0.34


def build_program(has_bqkv: bool, has_spb1: bool, has_spb2: bool):
    nc = bacc.Bacc()

    x_d = nc.dram_tensor("x", [128, NT, DIM], BF16, kind="ExternalInput")
    xyzv_d = nc.dram_tensor("xyzv", [128, NT, 4], F8, kind="ExternalInput")
    xyzt_d = nc.dram_tensor("xyzt", [4, TQ], BF16, kind="ExternalInput")
    featt_d = nc.dram_tensor("featt", [128, 2, TQ], F32, kind="ExternalInput")
    wqkv_d = nc.dram_tensor("wqkv", [128, 2, 3 * INNER], F8, kind="ExternalInput")
    spw1_d = nc.dram_tensor("spw1", [4, 2 * DIM], BF16, kind="ExternalInput")
    spw2_d = nc.dram_tensor("spw2", [128, 4, DH], F8, kind="ExternalInput")
    wout_d = nc.dram_tensor("wout", [64, H, DIM], F8, kind="ExternalInput")
    cf32_d = nc.dram_tensor("cf32", [128, 16], F32, kind="ExternalInput")
    cbf_d = nc.dram_tensor("cbf", [1, TQ + INNER + DH], BF16, kind="ExternalInput")
    out_d = nc.dram_tensor("out", [128, 2, TQ], F32, kind="ExternalOutput")

    with tile.TileContext(nc) as tc:
        with (
            tc.tile_pool(name="const", bufs=1) as constp,
            tc.tile_pool(name="big", bufs=1) as bigp,
            tc.tile_pool(name="work", bufs=2) as workp,
        ):
            # ---- DMAs: all on the sync HWDGE queue, critical-path first
            # (x group 0 leads so the LN chain starts as early as possible).
            x_sb = bigp.tile([128, NT, DIM], BF16)
            xv = x_d[:]
            nc.sync.dma_start(out=x_sb[:, 0:4, :], in_=xv[:, 0:4, :])
            wqkv_sb = constp.tile([128, 2, 3 * INNER], F8)
            nc.sync.dma_start(out=wqkv_sb, in_=wqkv_d[:])
            for g in range(1, 4):
                nc.sync.dma_start(
                    out=x_sb[:, 4 * g : 4 * g + 4, :],
                    in_=xv[:, 4 * g : 4 * g + 4, :],
                )
            xyzv_sb = constp.tile([128, NT, 4], F8)
            nc.sync.dma_start(out=xyzv_sb, in_=xyzv_d[:])
            xyzt_sb = constp.tile([4, TQ], BF16)
            nc.sync.dma_start(out=xyzt_sb, in_=xyzt_d[:])
            cbf_sb = constp.tile([1, TQ + INNER + DH], BF16)
            nc.sync.dma_start(out=cbf_sb, in_=cbf_d[:])
            cf32_sb = constp.tile([128, 16], F32)
            nc.sync.dma_start(out=cf32_sb, in_=cf32_d[:])
            spw1_sb = constp.tile([4, 2 * DIM], BF16)
            nc.sync.dma_start(out=spw1_sb, in_=spw1_d[:])
            spw2_sb = constp.tile([128, 4, DH], F8)
            nc.sync.dma_start(out=spw2_sb, in_=spw2_d[:])
            wout_sb = constp.tile([64, H, DIM], F8)
            nc.sync.dma_start(out=wout_sb, in_=wout_d[:])
            featt_sb = constp.tile([128, 2, TQ], F32)
            nc.sync.dma_start(out=featt_sb, in_=featt_d[:])

            ones_tq = cbf_sb[0:1, 0:TQ]
            bv_sb = cbf_sb[0:1, TQ : TQ + INNER]
            spb2_sb = cbf_sb[0:1, TQ + INNER : TQ + INNER + DH]
            bqk_sb = cf32_sb[:, 0:8]
            spb1_sb = cf32_sb[:, 8:12]
            outb_sb = cf32_sb[:, 12:14]

            ident = constp.tile([128, 128], BF16)
            make_identity(nc, ident)
            eps_t = constp.tile([128, 1], F32)
            nc.vector.memset(eps_t, LN_EPS)

            # xyz|ones columns of Vaug (pre-scaled by 1/32 on host); the
            # v and xyz parts live in separate tiles so each DoubleRow
            # stationary k-tile stride is a power of two.
            vv_sb = bigp.tile([128, NT, H, DH], F8)
            vx_sb = bigp.tile([128, NT, H, 4], F8)
            for h in range(H):
                nc.gpsimd.tensor_copy(vx_sb[:, :, h, :], xyzv_sb)

            # PE priming: absorb one DMA-queue semaphore per DMA-loaded
            # tile the PE consumes + warm spam for the HAM clock gate.
            pwarm_cm = tc.tile_pool(name="pwarm", bufs=1, space="PSUM")
            pwarm = pwarm_cm.__enter__()
            warm_ps = pwarm.tile([128, 128], BF16, tag="warm", bufs=1)

            def warm(n):
                for _ in range(n):
                    nc.tensor.transpose(warm_ps, ident, ident)

            warm(24)
            prime_ps = pwarm.tile([4, 4], F32, tag="prime", bufs=1)

            def prime(lhsT, rhs):
                nc.tensor.matmul(
                    prime_ps[0 : lhsT.shape[-1], 0 : rhs.shape[-1]],
                    lhsT,
                    rhs,
                    start=True,
                    stop=True,
                )

            prime(wqkv_sb[:, 0, 0:4], wqkv_sb[:, 0, 0:4])
            prime(spw1_sb[:, 0:4], spw1_sb[:, 0:4])
            prime(spw2_sb[:, 0, 0:4], spw2_sb[:, 0, 0:4])
            prime(wout_sb[:, 0, 0:4], wout_sb[:, 0, 0:4])
            if has_bqkv:
                prime(ones_tq[:, 0:4], bv_sb[:, 0:4])
            if has_spb2:
                prime(spb2_sb[:, 0:4], ones_tq[:, 0:4])
            warm(30)
            pwarm_cm.__exit__(None, None, None)

            # ---- Phase A: LN -> transpose -> q/k/v (plain fp8 matmuls),
            # pipelined per 4-tile group; LN stats for group g+1 issue
            # ahead of group g's evacuations so the ACT sqrt never queues
            # behind them.  All of k is emitted here (attention needs the
            # full 8 PSUM banks for itself).
            xn_sb = bigp.tile([128, NT, DIM], BF16)
            xnt_sb = bigp.tile([128, 2, M], F8)
            qt_sb = bigp.tile([128, 4, TQ], BF16)
            kt_sb = bigp.tile([128, 4, M], BF16)
            mv_all = constp.tile([128, NT, 2], F32)
            rstd = constp.tile([128, NT], F32)

            ptr_cm = tc.tile_pool(name="ptr", bufs=2, space="PSUM")
            ptr = ptr_cm.__enter__()
            pkq_cm = tc.tile_pool(name="pkq", bufs=2, space="PSUM")
            pkq = pkq_cm.__enter__()
            pv_cm = tc.tile_pool(name="pv", bufs=2, space="PSUM")
            pv = pv_cm.__enter__()

            def ln_stats(g):
                for q in range(4):
                    n = 4 * g + q
                    stats = workp.tile([128, 6], F32, tag="bnstats")
                    nc.vector.bn_stats(out=stats, in_=x_sb[:, n, :])
                    nc.vector.bn_aggr(out=mv_all[:, n, :], in_=stats)
                nc.scalar.activation(
                    out=rstd[:, 4 * g : 4 * g + 4],
                    in_=mv_all[:, 4 * g : 4 * g + 4, 1],
                    func=AF.Sqrt,
                    bias=eps_t,
                    scale=1.0,
                )

            def ln_recip(g):
                nc.vector.reciprocal(
                    out=rstd[:, 4 * g : 4 * g + 4],
                    in_=rstd[:, 4 * g : 4 * g + 4],
                )

            def emit_q():
                for grp in range(2):
                    ps_q = pkq.tile([128, 2, TQ], F32, tag="kq", bufs=2)
                    for oo in range(2):
                        oc = grp * 2 + oo
                        for cc in range(2):
                            nc.tensor.matmul(
                                ps_q[:, oo, :],
                                wqkv_sb[:, cc, oc * 128 : (oc + 1) * 128],
                                xnt_sb[:, cc, 0:TQ],
                                start=(cc == 0),
                                stop=(cc == 1),
                            )
                    if has_bqkv:
                        for oo in range(2):
                            oc = grp * 2 + oo
                            nc.vector.tensor_scalar(
                                out=qt_sb[:, oc, :],
                                in0=ps_q[:, oo, :],
                                scalar1=WQ_INV,
                                scalar2=bqk_sb[:, oc : oc + 1],
                                op0=OP.mult,
                                op1=OP.add,
                            )
                    else:
                        nc.vector.tensor_scalar(
                            out=qt_sb[:, grp * 2 : grp * 2 + 2, :],
                            in0=ps_q,
                            scalar1=WQ_INV,
                            scalar2=None,
                            op0=OP.mult,
                        )

            def emit_k(g):
                # only the oc0 chunk (heads 0/1, needed by pass 0) is
                # emitted in phase A; oc1..3 ride lazily inside attention
                # passes 0..2 in the slots' spare PE/ACT bandwidth.
                ps_k = pkq.tile([128, 2, TQ], F32, tag="kq", bufs=2)
                for cc in range(2):
                    nc.tensor.matmul(
                        ps_k[:, 0, :],
                        wqkv_sb[:, cc, INNER : INNER + 128],
                        xnt_sb[:, cc, g * TQ : (g + 1) * TQ],
                        start=(cc == 0),
                        stop=(cc == 1),
                    )
                if has_bqkv:
                    nc.vector.tensor_scalar(
                        out=kt_sb[:, 0, g * TQ : (g + 1) * TQ],
                        in0=ps_k[:, 0, :],
                        scalar1=WQ_INV,
                        scalar2=bqk_sb[:, 4:5],
                        op0=OP.mult,
                        op1=OP.add,
                    )
                else:
                    nc.scalar.activation(
                        out=kt_sb[:, 0, g * TQ : (g + 1) * TQ],
                        in_=ps_k[:, 0, :],
                        func=AF.Copy,
                        scale=WQ_INV,
                    )

            ln_stats(0)
            ln_recip(0)
            for g in range(4):
                if g + 1 < 4:
                    ln_stats(g + 1)
                for q in range(4):
                    n = 4 * g + q
                    nc.vector.tensor_scalar(
                        out=xn_sb[:, n, :],
                        in0=x_sb[:, n, :],
                        scalar1=mv_all[:, n, 0:1],
                        scalar2=rstd[:, n : n + 1],
                        op0=OP.subtract,
                        op1=OP.mult,
                    )
                # transpose this group into xnT (cast to fp8 at evac)
                for cc in range(2):
                    ps = ptr.tile([128, 512], BF16, tag="tr")
                    for q in range(4):
                        n = 4 * g + q
                        nc.tensor.transpose(
                            ps[:, q * 128 : (q + 1) * 128],
                            xn_sb[:, n, cc * 128 : (cc + 1) * 128],
                            ident,
                        )
                    if g < 2:
                        nc.scalar.activation(
                            out=xnt_sb[:, cc, g * 512 : (g + 1) * 512],
                            in_=ps,
                            func=AF.Copy,
                        )
                    else:
                        nc.vector.tensor_copy(
                            xnt_sb[:, cc, g * 512 : (g + 1) * 512], ps
                        )
                if g + 1 < 4:
                    ln_recip(g + 1)
                if g == 0:
                    emit_q()
                emit_k(g)
                # v for this group, evacuated on the scalar engine.
                for q in range(4):
                    n = 4 * g + q
                    ps_v = pv.tile([128, INNER], F32, tag="v", bufs=2)
                    for cc in range(2):
                        nc.tensor.matmul(
                            ps_v,
                            xnt_sb[:, cc, n * 128 : (n + 1) * 128],
                            wqkv_sb[:, cc, 2 * INNER : 3 * INNER],
                            start=(cc == 0),
                            stop=(cc == 1 and not has_bqkv),
                        )
                    if has_bqkv:
                        nc.tensor.matmul(
                            ps_v,
                            ones_tq[:, 0:128],
                            bv_sb,
                            start=False,
                            stop=True,
                            skip_group_check=True,
                        )
                    nc.scalar.activation(
                        out=vv_sb[:, n, :, :],
                        in_=ps_v[:].rearrange("p (h d) -> p h d", h=H),
                        func=AF.Copy,
                        scale=WQ_INV,
                    )

            pv_cm.__exit__(None, None, None)
            pkq_cm.__exit__(None, None, None)
            ptr_cm.__exit__(None, None, None)

            # ---- attention: 4 passes x 2 heads, j-tiles processed in
            # pairs so AV runs as fp8 DoubleRow (2 j-tiles per matmul).
            araw_sb = bigp.tile([64, 4, 2, TQ], F32)
            arax_sb = bigp.tile([4, 4, 2, TQ], F32)
            an_sb = bigp.tile([64, 4, 2, TQ], F32)
            # 4 rows, not 3: a 3-partition moving operand runs the PE at
            # half rate (4 is full rate).  Row 3 multiplies a zeroed
            # spw1 row, so it only needs to be finite: the dn chain
            # writes all 4 rows (row 3 = denominator*rbc, harmless).
            dnp_sb = bigp.tile([4, 4, 2, TQ], BF16)
            rsp_cm = tc.tile_pool(name="rsp", bufs=2)
            rsp = rsp_cm.__enter__()
            with (
                tc.tile_pool(name="pattn", bufs=2, space="PSUM") as pattn,
                tc.tile_pool(name="expp", bufs=2) as expp,
            ):
                def qk_one(p, j, hh):
                    sTh = pattn.tile([128, TQ], F32, tag="sT", bufs=4)
                    nc.tensor.matmul(
                        sTh,
                        kt_sb[hh * 64 : hh * 64 + 64, p, j * 128 : (j + 1) * 128],
                        qt_sb[hh * 64 : hh * 64 + 64, p, :],
                        start=True,
                        stop=True,
                    )
                    return sTh

                def e_view(e):
                    return e[:].bitcast(F8).rearrange(
                        "p a b (t two) -> p a b t two", two=2
                    )

                def exp_act(e, jt, s0, s1):
                    e8 = e_view(e)
                    nc.scalar.activation(out=e8[:, jt, 0, :, 0], in_=s0, func=AF.Exp)
                    nc.scalar.activation(out=e8[:, jt, 1, :, 0], in_=s1, func=AF.Exp)

                def exp_dve(e, jt, s0, s1):
                    for hh, s in ((0, s0), (1, s1)):
                        nc.vector.tensor_scalar(
                            out=e[:, jt, hh, :],
                            in0=s,
                            scalar1=EXP_A8,
                            scalar2=EXP_B8,
                            op0=OP.mult,
                            op1=OP.add,
                        )

                def qk_exp_pair(p, t):
                    # int16-spaced e buffer [128, jt, hh, i]: ACT writes
                    # exact-exp fp8 into the low bytes of the even tile,
                    # the DVE writes Schraudolph i16 (low byte = e4m3
                    # bits) into the odd tile at its 2x rate.  AV streams
                    # the stride-2 fp8 view.
                    s00 = qk_one(p, 2 * t, 0)
                    s01 = qk_one(p, 2 * t, 1)
                    s10 = qk_one(p, 2 * t + 1, 0)
                    s11 = qk_one(p, 2 * t + 1, 1)
                    e = expp.tile([128, 2, 2, TQ], I16, tag="e", bufs=3)
                    exp_act(e, 0, s00, s01)
                    exp_dve(e, 1, s10, s11)
                    return e

                def norm_evac(pp, acc_v, acc_x):
                    # arax on the scalar engine ahead of the next pass's
                    # exps (the xyz AV of slot 0 waits on it), araw on the
                    # DVE (the v AV runs later in the slot).
                    nc.scalar.activation(
                        out=arax_sb[:, pp, :, :], in_=acc_x, func=AF.Copy
                    )
                    nc.vector.tensor_copy(araw_sb[:, pp, :, :], acc_v)
                    rs = rsp.tile([128, 8], F32, tag="rs")
                    nc.sync.dma_start(out=rs, in_=arax_sb[3:4, pp, :, :])
                    rc = rsp.tile([128, 8], F32, tag="rc")
                    nc.vector.reciprocal(out=rc, in_=rs)
                    rrow = rsp.tile([1, 2, TQ], F32, tag="rrow")
                    nc.sync.dma_start(out=rrow, in_=rc)
                    return rrow

                def norm_rest(pp, rrow):
                    # rbc holds 32/denominator (the ones column is 1/32),
                    # so an = 32*attn@v and the xyz rows (pre-scaled 1/32)
                    # come out at true scale.
                    for hh in range(2):
                        rbc = rsp.tile([68, TQ], F32, tag="rbc", bufs=3)
                        nc.gpsimd.partition_broadcast(
                            rbc, rrow[0:1, hh, :], channels=68
                        )
                        nc.vector.tensor_tensor(
                            out=an_sb[:, pp, hh, :],
                            in0=araw_sb[:, pp, hh, :],
                            in1=rbc[0:64, :],
                            op=OP.mult,
                        )
                        dn = dnp_sb[:, pp, hh, :]
                        nc.vector.tensor_tensor(
                            out=dn,
                            in0=arax_sb[:, pp, hh, :],
                            in1=rbc[0:4, :],
                            op=OP.mult,
                        )
                        nc.vector.tensor_tensor(
                            out=dn,
                            in0=dn,
                            in1=xyzt_sb,
                            op=OP.subtract,
                        )

                prev = None
                for p in range(4):
                    acc_v = pattn.tile([64, 2, TQ], F32, tag="accv", bufs=1)
                    acc_x = pattn.tile([4, 2, TQ], F32, tag="accx", bufs=1)
                    if prev is not None:
                        prev_rrow = norm_evac(*prev)
                    e_cur = e_view(qk_exp_pair(p, 0))
                    for t in range(8):
                        if p < 3 and 2 <= t <= 5:
                            # lazy kt chunk oc=p+1, token group t-2
                            oc, g = p + 1, t - 2
                            kps = pattn.tile([128, TQ], F32, tag="sT", bufs=4)
                            for cc in range(2):
                                nc.tensor.matmul(
                                    kps,
                                    wqkv_sb[
                                        :,
                                        cc,
                                        INNER + oc * 128 : INNER + (oc + 1) * 128,
                                    ],
                                    xnt_sb[:, cc, g * TQ : (g + 1) * TQ],
                                    start=(cc == 0),
                                    stop=(cc == 1),
                                )
                            if has_bqkv:
                                nc.vector.tensor_scalar(
                                    out=kt_sb[:, oc, g * TQ : (g + 1) * TQ],
                                    in0=kps,
                                    scalar1=WQ_INV,
                                    scalar2=bqk_sb[:, 4 + oc : 5 + oc],
                                    op0=OP.mult,
                                    op1=OP.add,
                                )
                            else:
                                nc.scalar.activation(
                                    out=kt_sb[:, oc, g * TQ : (g + 1) * TQ],
                                    in_=kps,
                                    func=AF.Copy,
                                    scale=WQ_INV,
                                )
                        if t + 1 < 8:
                            e_nxt = qk_exp_pair(p, t + 1)
                            if p == 3 and t == 6:
                                # pull the Gelu ACT table in behind the
                                # final exps so the MLP doesn't pay it.
                                # Reads an_sb so the instruction scheduler
                                # cannot hoist it into earlier phases.
                                dg = workp.tile([64, 4], F32, tag="dg")
                                nc.scalar.activation(
                                    out=dg,
                                    in_=an_sb[:, 2, 0, 0:4],
                                    func=AF.Gelu,
                                )
                        else:
                            e_nxt = None
                        if t == 2 and prev is not None:
                            norm_rest(prev[0], prev_rrow)
                            prev = None
                        for hh in range(2):
                            h = 2 * p + hh
                            nc.tensor.matmul(
                                acc_x[:, hh, :],
                                vx_sb[:, 2 * t : 2 * t + 2, h, :],
                                e_cur[:, :, hh, :, 0],
                                start=(t == 0),
                                stop=(t == 7),
                                perf_mode=DR,
                            )
                        for hh in range(2):
                            h = 2 * p + hh
                            nc.tensor.matmul(
                                acc_v[:, hh, :],
                                vv_sb[:, 2 * t : 2 * t + 2, h, :],
                                e_cur[:, :, hh, :, 0],
                                start=(t == 0),
                                stop=(t == 7),
                                perf_mode=DR,
                            )
                        e_cur = e_view(e_nxt) if e_nxt is not None else None
                    if p == 3:
                        # keep the PE busy through the norm chain + pool
                        # handoff so HAM stays at full clock into the MLP.
                        wps = pattn.tile([128, TQ], F32, tag="sT", bufs=4)
                        wv = wps[:, 0:128].bitcast(BF16)[:, 0:128]
                        for _ in range(6):
                            nc.tensor.transpose(wv, ident, ident)
                    prev = (p, acc_v, acc_x)
                prev3 = prev
                prev3_rrow = norm_evac(*prev)
            # ---- spatial-bias MLP + out projection, pipelined per head:
            # h1 (bf16, kc pair) -> one gelu -> h2 (fp8 DoubleRow), then
            # outfin = an + sbias; out-proj (plain fp8) accumulates into
            # yT as soon as each head pair completes.
            outfin_sb = bigp.tile([64, H, TQ], F8)
            with (
                tc.tile_pool(name="pmlp", bufs=1, space="PSUM") as pmlp,
                tc.tile_pool(name="hpool", bufs=2) as hpool,
            ):
                yT = pmlp.tile([128, 2, TQ], F32, tag="yT", bufs=1)
                wv = yT[:, 0, 0:64].bitcast(BF16)
                for _ in range(30):
                    nc.tensor.transpose(wv, ident, ident)

                # flat (head, kc-pair) pipeline with one-iteration h1
                # lookahead: the next h1 pair issues BEFORE h2 of the
                # current one, so the in-order PE queue never head-blocks
                # on a gelu that is still streaming.
                items = [(m, hh, kcp) for m in range(4) for hh in range(2)
                         for kcp in range(2)]

                def issue_h1(i):
                    m, hh, kcp = items[i]
                    h1 = pmlp.tile([128, 2, TQ], F32, tag="h1", bufs=2)
                    for kk in range(2):
                        kc = 2 * kcp + kk
                        nc.tensor.matmul(
                            h1[:, kk, :],
                            spw1_sb[:, kc * 128 : (kc + 1) * 128],
                            dnp_sb[:, m, hh, :],
                            start=True,
                            stop=True,
                        )
                    return h1

                h1_next = issue_h1(0)
                sb_t = None
                for i, (m, hh, kcp) in enumerate(items):
                    h = 2 * m + hh
                    if kcp == 0:
                        sb_t = pmlp.tile([64, TQ], F32, tag="sb", bufs=2)
                    h1 = h1_next
                    if i + 1 < len(items):
                        h1_next = issue_h1(i + 1)
                    hsb = hpool.tile([128, 2, TQ], F8, tag="hsb", bufs=3)
                    if has_spb1:
                        for kk in range(2):
                            kc = 2 * kcp + kk
                            nc.scalar.activation(
                                out=hsb[:, kk, :],
                                in_=h1[:, kk, :],
                                func=AF.Gelu,
                                bias=spb1_sb[:, kc : kc + 1],
                            )
                    else:
                        nc.scalar.activation(out=hsb, in_=h1, func=AF.Gelu)
                    nc.tensor.matmul(
                        sb_t,
                        spw2_sb[:, 2 * kcp : 2 * kcp + 2, :],
                        hsb,
                        start=(kcp == 0),
                        stop=(kcp == 1 and not has_spb2),
                        perf_mode=DR,
                    )
                    if kcp == 1:
                        if has_spb2:
                            nc.tensor.matmul(
                                sb_t,
                                spb2_sb,
                                ones_tq,
                                start=False,
                                stop=True,
                                skip_group_check=True,
                            )
                        nc.vector.tensor_tensor(
                            out=outfin_sb[:, h, :],
                            in0=an_sb[:, m, hh, :],
                            in1=sb_t,
                            op=OP.add,
                        )
                        for ec in range(2):
                            nc.tensor.matmul(
                                yT[:, ec, :],
                                wout_sb[:, h, ec * 128 : (ec + 1) * 128],
                                outfin_sb[:, h, :],
                                start=(h == 0),
                                stop=(h == H - 1),
                            )
                    if i == 7:
                        # pass-3 normalization arithmetic lands here so
                        # heads 0..3's outfin adds aren't queued behind
                        # it on the DVE (heads 6/7 need it from i=12).
                        norm_rest(prev3[0], prev3_rrow)

                # ---- final gelu (fused 2^-11 descale) + residual ----
                for ec in range(2):
                    ysb = workp.tile([128, TQ], F32, tag="ysb")
                    nc.scalar.activation(
                        out=ysb,
                        in_=yT[:, ec, :],
                        func=AF.Gelu,
                        bias=outb_sb[:, ec : ec + 1],
                        scale=YT_INV,
                    )
                    res = workp.tile([128, TQ], F32, tag="res")
                    nc.vector.tensor_tensor(
                        out=res, in0=ysb, in1=featt_sb[:, ec, :], op=OP.add
                    )
                    nc.sync.dma_start(out=out_d[:, ec, :], in_=res)
            rsp_cm.__exit__(None, None, None)

    nc.compile()
    return nc


def prepare_maps(inputs):
    xyzs = np.asarray(inputs["xyzs"], np.float32)
    features = np.asarray(inputs["features"], np.float32)
    ln_g = np.asarray(inputs["ln_g"], np.float32)
    ln_b = np.asarray(inputs["ln_b"], np.float32)
    w_qkv = np.asarray(inputs["w_qkv"], np.float32)
    sp_w1 = np.asarray(inputs["sp_w1"], np.float32)
    sp_b1 = np.asarray(inputs["sp_b1"], np.float32)
    sp_w2 = np.asarray(inputs["sp_w2"], np.float32)
    sp_b2 = np.asarray(inputs["sp_b2"], np.float32)
    out_w = np.asarray(inputs["out_w"], np.float32)
    out_b = np.asarray(inputs["out_b"], np.float32)

    scale = DH ** -0.5
    wqkv_f = w_qkv * ln_g[:, None]
    wqkv_f[:, :INNER] = wqkv_f[:, :INNER] * scale
    bqkv = (ln_b @ w_qkv).astype(np.float32)
    bqkv[:INNER] *= scale

    has_bqkv = bool(np.any(bqkv != 0.0))
    has_spb1 = bool(np.any(sp_b1 != 0.0))
    has_spb2 = bool(np.any(sp_b2 != 0.0))

    cf32 = np.zeros((128, 16), np.float32)
    for oc in range(4):
        cf32[:, oc] = bqkv[oc * 128 : (oc + 1) * 128]
        cf32[:, 4 + oc] = bqkv[INNER + oc * 128 : INNER + (oc + 1) * 128]
    for kc in range(4):
        cf32[:, 8 + kc] = sp_b1[kc * 128 : (kc + 1) * 128]
    cf32[:, 12] = out_b[:128]
    cf32[:, 13] = out_b[128:]

    cbf = np.zeros((1, TQ + INNER + DH), np.float32)
    cbf[0, 0:TQ] = 1.0
    cbf[0, TQ : TQ + INNER] = bqkv[2 * INNER :] * WQ_S
    cbf[0, TQ + INNER :] = sp_b2 * W2_S

    # wout as [64, H, 256]: row (d, h) = out_w[h*64+d, :]
    wout64 = np.ascontiguousarray(out_w.reshape(H, 64, DIM).transpose(1, 0, 2))

    wqkv8 = (wqkv_f * WQ_S).reshape(2, 128, 3 * INNER).transpose(1, 0, 2)
    spw232 = (sp_w2 * W2_S).reshape(4, 128, DH).transpose(1, 0, 2)
    shared = {
        "wqkv": np.ascontiguousarray(wqkv8).astype(E4),
        "cf32": cf32,
        "cbf": cbf.astype(BF),
        "spw1": np.ascontiguousarray(
            np.concatenate([sp_w1, np.zeros((1, 2 * DIM), np.float32)])
        ).astype(BF),
        "spw2": np.ascontiguousarray(spw232).astype(E4),
        "wout": (wout64 * WO_S).astype(E4),
    }

    in_maps = []
    for core in range(N_CORES):
        bi, quarter = core // 4, core % 4
        qs = quarter * TQ
        x_b = features[bi].reshape(M, DIM)
        xyz_b = xyzs[bi].reshape(M, 3)
        x_perm = np.roll(x_b, -qs, axis=0)
        xyz_perm = np.roll(xyz_b, -qs, axis=0)
        xyza = np.concatenate(
            [xyz_perm / OF_S, np.full((M, 1), 1.0 / OF_S, np.float32)], axis=1
        ).astype(np.float32)
        m = dict(shared)
        m["x"] = np.ascontiguousarray(
            x_perm.reshape(NT, 128, DIM).transpose(1, 0, 2)
        ).astype(BF)
        m["xyzv"] = np.ascontiguousarray(
            xyza.reshape(NT, 128, 4).transpose(1, 0, 2)
        ).astype(E4)
        m["xyzt"] = np.ascontiguousarray(
            np.concatenate([xyz_perm[:TQ].T, np.zeros((1, TQ), np.float32)])
        ).astype(BF)
        m["featt"] = np.ascontiguousarray(
            x_perm[:TQ].T.reshape(2, 128, TQ).transpose(1, 0, 2)
        )
        in_maps.append(m)
    return in_maps, (has_bqkv, has_spb1, has_spb2)


def assemble(results, l=16, n=128):
    out = np.zeros((2, M, DIM), np.float32)
    for core in range(N_CORES):
        bi, quarter = core // 4, core % 4
        qs = quarter * TQ
        o = results[core]["out"]  # [128, 2, TQ]
        out[bi, qs : qs + TQ, :] = (
            o.transpose(1, 0, 2).reshape(DIM, TQ).T
        )
    return out.reshape(2, l, n, DIM)


def kernel(**inputs):
    in_maps, flags = prepare_maps(inputs)
    nc = build_program(*flags)
    results = run_bass_kernel_spmd(nc, in_maps, list(range(N_CORES))).results
    return assemble(results)


if __name__ == "__main__":
    pass


# revision 25
# speedup vs baseline: 1.0075x; 1.0075x over previous
"""Trainium2 Bass kernel for nn_Attention_1322849927460.

Dense transformer block: LN -> qkv -> attention (+ spatial-bias MLP on
attention-weighted coordinate deltas) -> out proj -> gelu -> residual.

Sharding: 8 cores = (2 batches) x (4 sequence quarters). Each core holds
all 8 heads for its 512 query rows and the full 2048-token K/V of its
batch, so no collectives are needed. A host-side roll of the token axis
puts each core's query rows first, letting all cores run an identical
SPMD program (attention is invariant to key-order permutation).

Algebraic structure:
  * delta_full[b,h,i,:] = (attn @ xyz)[b,h,i,:] - xyz[b,i,:] since softmax
    rows sum to one -> the (m,m,3) delta tensor is never formed.
  * softmax denominators come free from augmented V' columns [xyz/32, 1/32];
    one reciprocal + partition-broadcast normalizes the accumulators.  The
    1/32 ones-column also lands the normalized attention output at 32x
    true scale, lifting the fp8 outfin tensor out of e4m3 denormals free.
  * ln_g and the 1/sqrt(dh) q-scale fold into the qkv weights on host.

fp8 strategy (vs the bf16 baseline):
  * all projection matmuls run in fp8e4 (same column cost as bf16, but
    half the weight DMA); AV runs as fp8 DoubleRow over adjacent j-tile
    pairs (2 k-tiles per instruction, halving AV instruction count), with
    the xyz/ones columns as a second tiny DoubleRow into a base-0 [4, .]
    accumulator (DoubleRow outputs must start at partition 0, and the
    stationary k-tile stride must be a power of two - walrus crashes
    otherwise, hence split vv/vx tiles).  Spatial-MLP h2 is DoubleRow
    over kc pairs.  QK and MLP h1 stay bf16 (PSUM-output-bound).
  * weights quantized with power-of-2 host scales (wqkv x64, spw2 x32,
    wout x64), descaled for free inside evacuation ops / the final ACT.
  * exp alternates per j-tile: even tiles exact on the scalar engine,
    odd tiles on the DVE via Schraudolph into e4m3 bits.  Both write an
    int16-spaced e buffer (DVE needs a 2-byte output dtype for its 2x
    mode; i16 = round(x*8/ln2 + 55.66) has the e4m3 bit pattern in its
    low byte) and AV reads a stride-2 fp8 view.
"""

import os
import sys

for _p in ("/opt/trn_rl_repo",):
    if _p not in sys.path and os.path.isdir(_p):
        sys.path.insert(0, _p)

import ml_dtypes
import numpy as np

import concourse.bass as bass
import concourse.bacc as bacc
import concourse.tile as tile
from concourse import mybir
from concourse.bass_utils import run_bass_kernel_spmd
from concourse.masks import make_identity

F32 = mybir.dt.float32
BF16 = mybir.dt.bfloat16
F8 = mybir.dt.float8e4
I16 = mybir.dt.int16
AF = mybir.ActivationFunctionType
OP = mybir.AluOpType
DR = mybir.MatmulPerfMode.DoubleRow
BF = ml_dtypes.bfloat16
E4 = ml_dtypes.float8_e4m3

DIM = 256
H = 8
DH = 64
INNER = H * DH  # 512
M = 2048  # tokens per batch
TQ = 512  # query tokens per core
NT = M // 128  # 16 token tiles
N_CORES = 8
LN_EPS = 1e-5

# host-side fp8 weight scales (powers of two; descaled on-chip for free)
WQ_S = 64.0      # wqkv scale; descale 2^-6 in the q/k/v evacuations
W2_S = 32.0      # spw2 scale == the outfin x32 target scale
WO_S = 64.0      # wout scale; total descale 2^-11 in the final gelu
OF_S = 32.0      # outfin scale, produced by the 1/32 ones/xyz columns
WQ_INV = 1.0 / WQ_S
YT_INV = 1.0 / (OF_S * WO_S)

# Schraudolph fast exp in e4m3 bit domain:
#   e4m3(x) bits = round(x * 2^3/ln2 + (7*2^3 - 0.34)) viewed as int8.
EXP_A8 = 8.0 / float(np.log(2.0))
EXP_B8 = 56.0 - 0.34


def build_program(has_bqkv: bool, has_spb1: bool, has_spb2: bool):
    nc = bacc.Bacc()

    x_d = nc.dram_tensor("x", [128, NT, DIM], BF16, kind="ExternalInput")
    xyzv_d = nc.dram_tensor("xyzv", [128, NT, 4], F8, kind="ExternalInput")
    xyzt_d = nc.dram_tensor("xyzt", [4, TQ], BF16, kind="ExternalInput")
    featt_d = nc.dram_tensor("featt", [128, 2, TQ], F32, kind="ExternalInput")
    wqkv_d = nc.dram_tensor("wqkv", [128, 2, 3 * INNER], F8, kind="ExternalInput")
    spw1_d = nc.dram_tensor("spw1", [4, 2 * DIM], BF16, kind="ExternalInput")
    spw2_d = nc.dram_tensor("spw2", [128, 4, DH], F8, kind="ExternalInput")
    wout_d = nc.dram_tensor("wout", [64, H, DIM], F8, kind="ExternalInput")
    cf32_d = nc.dram_tensor("cf32", [128, 16], F32, kind="ExternalInput")
    cbf_d = nc.dram_tensor("cbf", [1, TQ + INNER + DH], BF16, kind="ExternalInput")
    out_d = nc.dram_tensor("out", [128, 2, TQ], F32, kind="ExternalOutput")

    with tile.TileContext(nc) as tc:
        with (
            tc.tile_pool(name="const", bufs=1) as constp,
            tc.tile_pool(name="big", bufs=1) as bigp,
            tc.tile_pool(name="work", bufs=2) as workp,
        ):
            # ---- DMAs: all on the sync HWDGE queue, critical-path first
            # (x group 0 leads so the LN chain starts as early as possible).
            x_sb = bigp.tile([128, NT, DIM], BF16)
            xv = x_d[:]
            nc.sync.dma_start(out=x_sb[:, 0:4, :], in_=xv[:, 0:4, :])
            wqkv_sb = constp.tile([128, 2, 3 * INNER], F8)
            nc.sync.dma_start(out=wqkv_sb, in_=wqkv_d[:])
            for g in range(1, 4):
                nc.sync.dma_start(
                    out=x_sb[:, 4 * g : 4 * g + 4, :],
                    in_=xv[:, 4 * g : 4 * g + 4, :],
                )
            xyzv_sb = constp.tile([128, NT, 4], F8)
            nc.sync.dma_start(out=xyzv_sb, in_=xyzv_d[:])
            xyzt_sb = constp.tile([4, TQ], BF16)
            nc.sync.dma_start(out=xyzt_sb, in_=xyzt_d[:])
            cbf_sb = constp.tile([1, TQ + INNER + DH], BF16)
            nc.sync.dma_start(out=cbf_sb, in_=cbf_d[:])
            cf32_sb = constp.tile([128, 16], F32)
            nc.sync.dma_start(out=cf32_sb, in_=cf32_d[:])
            spw1_sb = constp.tile([4, 2 * DIM], BF16)
            nc.sync.dma_start(out=spw1_sb, in_=spw1_d[:])
            spw2_sb = constp.tile([128, 4, DH], F8)
            nc.sync.dma_start(out=spw2_sb, in_=spw2_d[:])
            wout_sb = constp.tile([64, H, DIM], F8)
            nc.sync.dma_start(out=wout_sb, in_=wout_d[:])
            featt_sb = constp.tile([128, 2, TQ], F32)
            nc.sync.dma_start(out=featt_sb, in_=featt_d[:])

            ones_tq = cbf_sb[0:1, 0:TQ]
            bv_sb = cbf_sb[0:1, TQ : TQ + INNER]
            spb2_sb = cbf_sb[0:1, TQ + INNER : TQ + INNER + DH]
            bqk_sb = cf32_sb[:, 0:8]
            spb1_sb = cf32_sb[:, 8:12]
            outb_sb = cf32_sb[:, 12:14]

            ident = constp.tile([128, 128], BF16)
            make_identity(nc, ident)
            eps_t = constp.tile([128, 1], F32)
            nc.vector.memset(eps_t, LN_EPS)

            # xyz|ones columns of Vaug (pre-scaled by 1/32 on host); the
            # v and xyz parts live in separate tiles so each DoubleRow
            # stationary k-tile stride is a power of two.
            vv_sb = bigp.tile([128, NT, H, DH], F8)
            vx_sb = bigp.tile([128, NT, H, 4], F8)
            for h in range(H):
                nc.gpsimd.tensor_copy(vx_sb[:, :, h, :], xyzv_sb)

            # PE priming: absorb one DMA-queue semaphore per DMA-loaded
            # tile the PE consumes + warm spam for the HAM clock gate.
            pwarm_cm = tc.tile_pool(name="pwarm", bufs=1, space="PSUM")
            pwarm = pwarm_cm.__enter__()
            warm_ps = pwarm.tile([128, 128], BF16, tag="warm", bufs=1)

            def warm(n):
                for _ in range(n):
                    nc.tensor.transpose(warm_ps, ident, ident)

            warm(24)
            prime_ps = pwarm.tile([4, 4], F32, tag="prime", bufs=1)

            def prime(lhsT, rhs):
                nc.tensor.matmul(
                    prime_ps[0 : lhsT.shape[-1], 0 : rhs.shape[-1]],
                    lhsT,
                    rhs,
                    start=True,
                    stop=True,
                )

            prime(wqkv_sb[:, 0, 0:4], wqkv_sb[:, 0, 0:4])
            prime(spw1_sb[:, 0:4], spw1_sb[:, 0:4])
            prime(spw2_sb[:, 0, 0:4], spw2_sb[:, 0, 0:4])
            prime(wout_sb[:, 0, 0:4], wout_sb[:, 0, 0:4])
            if has_bqkv:
                prime(ones_tq[:, 0:4], bv_sb[:, 0:4])
            if has_spb2:
                prime(spb2_sb[:, 0:4], ones_tq[:, 0:4])
            warm(30)
            pwarm_cm.__exit__(None, None, None)

            # ---- Phase A: LN -> transpose -> q/k/v (plain fp8 matmuls),
            # pipelined per 4-tile group; LN stats for group g+1 issue
            # ahead of group g's evacuations so the ACT sqrt never queues
            # behind them.  All of k is emitted here (attention needs the
            # full 8 PSUM banks for itself).
            xn_sb = bigp.tile([128, NT, DIM], BF16)
            xnt_sb = bigp.tile([128, 2, M], F8)
            qt_sb = bigp.tile([128, 4, TQ], BF16)
            kt_sb = bigp.tile([128, 4, M], BF16)
            mv_all = constp.tile([128, NT, 2], F32)
            rstd = constp.tile([128, NT], F32)

            ptr_cm = tc.tile_pool(name="ptr", bufs=2, space="PSUM")
            ptr = ptr_cm.__enter__()
            pkq_cm = tc.tile_pool(name="pkq", bufs=2, space="PSUM")
            pkq = pkq_cm.__enter__()
            pv_cm = tc.tile_pool(name="pv", bufs=2, space="PSUM")
            pv = pv_cm.__enter__()

            def ln_stats(g):
                for q in range(4):
                    n = 4 * g + q
                    stats = workp.tile([128, 6], F32, tag="bnstats")
                    nc.vector.bn_stats(out=stats, in_=x_sb[:, n, :])
                    nc.vector.bn_aggr(out=mv_all[:, n, :], in_=stats)
                nc.scalar.activation(
                    out=rstd[:, 4 * g : 4 * g + 4],
                    in_=mv_all[:, 4 * g : 4 * g + 4, 1],
                    func=AF.Sqrt,
                    bias=eps_t,
                    scale=1.0,
                )

            def ln_recip(g):
                nc.vector.reciprocal(
                    out=rstd[:, 4 * g : 4 * g + 4],
                    in_=rstd[:, 4 * g : 4 * g + 4],
                )

            def emit_q():
                for grp in range(2):
                    ps_q = pkq.tile([128, 2, TQ], F32, tag="kq", bufs=2)
                    for oo in range(2):
                        oc = grp * 2 + oo
                        for cc in range(2):
                            nc.tensor.matmul(
                                ps_q[:, oo, :],
                                wqkv_sb[:, cc, oc * 128 : (oc + 1) * 128],
                                xnt_sb[:, cc, 0:TQ],
                                start=(cc == 0),
                                stop=(cc == 1),
                            )
                    if has_bqkv:
                        for oo in range(2):
                            oc = grp * 2 + oo
                            nc.vector.tensor_scalar(
                                out=qt_sb[:, oc, :],
                                in0=ps_q[:, oo, :],
                                scalar1=WQ_INV,
                                scalar2=bqk_sb[:, oc : oc + 1],
                                op0=OP.mult,
                                op1=OP.add,
                            )
                    else:
                        nc.vector.tensor_scalar(
                            out=qt_sb[:, grp * 2 : grp * 2 + 2, :],
                            in0=ps_q,
                            scalar1=WQ_INV,
                            scalar2=None,
                            op0=OP.mult,
                        )

            def emit_k(g):
                # only the oc0 chunk (heads 0/1, needed by pass 0) is
                # emitted in phase A; oc1..3 ride lazily inside attention
                # passes 0..2 in the slots' spare PE/ACT bandwidth.
                ps_k = pkq.tile([128, 2, TQ], F32, tag="kq", bufs=2)
                for cc in range(2):
                    nc.tensor.matmul(
                        ps_k[:, 0, :],
                        wqkv_sb[:, cc, INNER : INNER + 128],
                        xnt_sb[:, cc, g * TQ : (g + 1) * TQ],
                        start=(cc == 0),
                        stop=(cc == 1),
                    )
                if has_bqkv:
                    nc.vector.tensor_scalar(
                        out=kt_sb[:, 0, g * TQ : (g + 1) * TQ],
                        in0=ps_k[:, 0, :],
                        scalar1=WQ_INV,
                        scalar2=bqk_sb[:, 4:5],
                        op0=OP.mult,
                        op1=OP.add,
                    )
                else:
                    nc.scalar.activation(
                        out=kt_sb[:, 0, g * TQ : (g + 1) * TQ],
                        in_=ps_k[:, 0, :],
                        func=AF.Copy,
                        scale=WQ_INV,
                    )

            ln_stats(0)
            ln_recip(0)
            for g in range(4):
                wtr = ptr.tile([128, 512], BF16, tag="tr")
                for _ in range(6):
                    nc.tensor.transpose(wtr[:, 0:128], ident, ident)
                if g + 1 < 4:
                    ln_stats(g + 1)
                for q in range(4):
                    n = 4 * g + q
                    nc.vector.tensor_scalar(
                        out=xn_sb[:, n, :],
                        in0=x_sb[:, n, :],
                        scalar1=mv_all[:, n, 0:1],
                        scalar2=rstd[:, n : n + 1],
                        op0=OP.subtract,
                        op1=OP.mult,
                    )
                # transpose this group into xnT (cast to fp8 at evac)
                for cc in range(2):
                    ps = ptr.tile([128, 512], BF16, tag="tr")
                    for q in range(4):
                        n = 4 * g + q
                        nc.tensor.transpose(
                            ps[:, q * 128 : (q + 1) * 128],
                            xn_sb[:, n, cc * 128 : (cc + 1) * 128],
                            ident,
                        )
                    if g < 2:
                        nc.scalar.activation(
                            out=xnt_sb[:, cc, g * 512 : (g + 1) * 512],
                            in_=ps,
                            func=AF.Copy,
                        )
                    else:
                        nc.vector.tensor_copy(
                            xnt_sb[:, cc, g * 512 : (g + 1) * 512], ps
                        )
                if g + 1 < 4:
                    ln_recip(g + 1)
                if g == 0:
                    emit_q()
                emit_k(g)
                # v for this group, evacuated on the scalar engine.
                for q in range(4):
                    n = 4 * g + q
                    ps_v = pv.tile([128, INNER], F32, tag="v", bufs=2)
                    for cc in range(2):
                        nc.tensor.matmul(
                            ps_v,
                            xnt_sb[:, cc, n * 128 : (n + 1) * 128],
                            wqkv_sb[:, cc, 2 * INNER : 3 * INNER],
                            start=(cc == 0),
                            stop=(cc == 1 and not has_bqkv),
                        )
                    if has_bqkv:
                        nc.tensor.matmul(
                            ps_v,
                            ones_tq[:, 0:128],
                            bv_sb,
                            start=False,
                            stop=True,
                            skip_group_check=True,
                        )
                    nc.scalar.activation(
                        out=vv_sb[:, n, :, :],
                        in_=ps_v[:].rearrange("p (h d) -> p h d", h=H),
                        func=AF.Copy,
                        scale=WQ_INV,
                    )

            pv_cm.__exit__(None, None, None)
            pkq_cm.__exit__(None, None, None)
            ptr_cm.__exit__(None, None, None)

            # ---- attention: 4 passes x 2 heads, j-tiles processed in
            # pairs so AV runs as fp8 DoubleRow (2 j-tiles per matmul).
            araw_sb = bigp.tile([64, 4, 2, TQ], F32)
            arax_sb = bigp.tile([4, 4, 2, TQ], F32)
            an_sb = bigp.tile([64, 4, 2, TQ], F32)
            # 4 rows, not 3: a 3-partition moving operand runs the PE at
            # half rate (4 is full rate).  Row 3 multiplies a zeroed
            # spw1 row, so it only needs to be finite: the dn chain
            # writes all 4 rows (row 3 = denominator*rbc, harmless).
            dnp_sb = bigp.tile([4, 4, 2, TQ], BF16)
            rsp_cm = tc.tile_pool(name="rsp", bufs=2)
            rsp = rsp_cm.__enter__()
            with (
                tc.tile_pool(name="pattn", bufs=2, space="PSUM") as pattn,
                tc.tile_pool(name="expp", bufs=2) as expp,
            ):
                def qk_one(p, j, hh):
                    sTh = pattn.tile([128, TQ], F32, tag="sT", bufs=4)
                    nc.tensor.matmul(
                        sTh,
                        kt_sb[hh * 64 : hh * 64 + 64, p, j * 128 : (j + 1) * 128],
                        qt_sb[hh * 64 : hh * 64 + 64, p, :],
                        start=True,
                        stop=True,
                    )
                    return sTh

                def e_view(e):
                    return e[:].bitcast(F8).rearrange(
                        "p a b (t two) -> p a b t two", two=2
                    )

                def exp_act(e, jt, s0, s1):
                    e8 = e_view(e)
                    nc.scalar.activation(out=e8[:, jt, 0, :, 0], in_=s0, func=AF.Exp)
                    nc.scalar.activation(out=e8[:, jt, 1, :, 0], in_=s1, func=AF.Exp)

                def exp_dve(e, jt, s0, s1):
                    for hh, s in ((0, s0), (1, s1)):
                        nc.vector.tensor_scalar(
                            out=e[:, jt, hh, :],
                            in0=s,
                            scalar1=EXP_A8,
                            scalar2=EXP_B8,
                            op0=OP.mult,
                            op1=OP.add,
                        )

                def qk_exp_pair(p, t):
                    # int16-spaced e buffer [128, jt, hh, i]: ACT writes
                    # exact-exp fp8 into the low bytes of the even tile,
                    # the DVE writes Schraudolph i16 (low byte = e4m3
                    # bits) into the odd tile at its 2x rate.  AV streams
                    # the stride-2 fp8 view.
                    s00 = qk_one(p, 2 * t, 0)
                    s01 = qk_one(p, 2 * t, 1)
                    s10 = qk_one(p, 2 * t + 1, 0)
                    s11 = qk_one(p, 2 * t + 1, 1)
                    e = expp.tile([128, 2, 2, TQ], I16, tag="e", bufs=3)
                    exp_act(e, 0, s00, s01)
                    exp_dve(e, 1, s10, s11)
                    return e

                def norm_evac(pp, acc_v, acc_x):
                    # arax on the scalar engine ahead of the next pass's
                    # exps (the xyz AV of slot 0 waits on it), araw on the
                    # DVE (the v AV runs later in the slot).
                    nc.scalar.activation(
                        out=arax_sb[:, pp, :, :], in_=acc_x, func=AF.Copy
                    )
                    nc.vector.tensor_copy(araw_sb[:, pp, :, :], acc_v)
                    rs = rsp.tile([128, 8], F32, tag="rs")
                    nc.sync.dma_start(out=rs, in_=arax_sb[3:4, pp, :, :])
                    rc = rsp.tile([128, 8], F32, tag="rc")
                    nc.vector.reciprocal(out=rc, in_=rs)
                    rrow = rsp.tile([1, 2, TQ], F32, tag="rrow")
                    nc.sync.dma_start(out=rrow, in_=rc)
                    return rrow

                def norm_rest(pp, rrow):
                    # rbc holds 32/denominator (the ones column is 1/32),
                    # so an = 32*attn@v and the xyz rows (pre-scaled 1/32)
                    # come out at true scale.
                    for hh in range(2):
                        rbc = rsp.tile([68, TQ], F32, tag="rbc", bufs=3)
                        nc.gpsimd.partition_broadcast(
                            rbc, rrow[0:1, hh, :], channels=68
                        )
                        nc.vector.tensor_tensor(
                            out=an_sb[:, pp, hh, :],
                            in0=araw_sb[:, pp, hh, :],
                            in1=rbc[0:64, :],
                            op=OP.mult,
                        )
                        dn = dnp_sb[:, pp, hh, :]
                        nc.vector.tensor_tensor(
                            out=dn,
                            in0=arax_sb[:, pp, hh, :],
                            in1=rbc[0:4, :],
                            op=OP.mult,
                        )
                        nc.vector.tensor_tensor(
                            out=dn,
                            in0=dn,
                            in1=xyzt_sb,
                            op=OP.subtract,
                        )

                prev = None
                for p in range(4):
                    acc_v = pattn.tile([64, 2, TQ], F32, tag="accv", bufs=1)
                    acc_x = pattn.tile([4, 2, TQ], F32, tag="accx", bufs=1)
                    if prev is not None:
                        prev_rrow = norm_evac(*prev)
                    e_cur = e_view(qk_exp_pair(p, 0))
                    for t in range(8):
                        if p < 3 and 2 <= t <= 5:
                            # lazy kt chunk oc=p+1, token group t-2
                            oc, g = p + 1, t - 2
                            kps = pattn.tile([128, TQ], F32, tag="sT", bufs=4)
                            for cc in range(2):
                                nc.tensor.matmul(
                                    kps,
                                    wqkv_sb[
                                        :,
                                        cc,
                                        INNER + oc * 128 : INNER + (oc + 1) * 128,
                                    ],
                                    xnt_sb[:, cc, g * TQ : (g + 1) * TQ],
                                    start=(cc == 0),
                                    stop=(cc == 1),
                                )
                            if has_bqkv:
                                nc.vector.tensor_scalar(
                                    out=kt_sb[:, oc, g * TQ : (g + 1) * TQ],
                                    in0=kps,
                                    scalar1=WQ_INV,
                                    scalar2=bqk_sb[:, 4 + oc : 5 + oc],
                                    op0=OP.mult,
                                    op1=OP.add,
                                )
                            else:
                                nc.scalar.activation(
                                    out=kt_sb[:, oc, g * TQ : (g + 1) * TQ],
                                    in_=kps,
                                    func=AF.Copy,
                                    scale=WQ_INV,
                                )
                        if t + 1 < 8:
                            e_nxt = qk_exp_pair(p, t + 1)
                            if p == 3 and t == 6:
                                # pull the Gelu ACT table in behind the
                                # final exps so the MLP doesn't pay it.
                                # Reads an_sb so the instruction scheduler
                                # cannot hoist it into earlier phases.
                                dg = workp.tile([64, 4], F32, tag="dg")
                                nc.scalar.activation(
                                    out=dg,
                                    in_=an_sb[:, 2, 0, 0:4],
                                    func=AF.Gelu,
                                )
                        else:
                            e_nxt = None
                        if t == 2 and prev is not None:
                            norm_rest(prev[0], prev_rrow)
                            prev = None
                        for hh in range(2):
                            h = 2 * p + hh
                            nc.tensor.matmul(
                                acc_x[:, hh, :],
                                vx_sb[:, 2 * t : 2 * t + 2, h, :],
                                e_cur[:, :, hh, :, 0],
                                start=(t == 0),
                                stop=(t == 7),
                                perf_mode=DR,
                            )
                        for hh in range(2):
                            h = 2 * p + hh
                            nc.tensor.matmul(
                                acc_v[:, hh, :],
                                vv_sb[:, 2 * t : 2 * t + 2, h, :],
                                e_cur[:, :, hh, :, 0],
                                start=(t == 0),
                                stop=(t == 7),
                                perf_mode=DR,
                            )
                        e_cur = e_view(e_nxt) if e_nxt is not None else None
                    if p == 3:
                        # keep the PE busy through the norm chain + pool
                        # handoff so HAM stays at full clock into the MLP.
                        wps = pattn.tile([128, TQ], F32, tag="sT", bufs=4)
                        wv = wps[:, 0:128].bitcast(BF16)[:, 0:128]
                        for _ in range(6):
                            nc.tensor.transpose(wv, ident, ident)
                    prev = (p, acc_v, acc_x)
                prev3 = prev
                prev3_rrow = norm_evac(*prev)
            # ---- spatial-bias MLP + out projection, pipelined per head:
            # h1 (bf16, kc pair) -> one gelu -> h2 (fp8 DoubleRow), then
            # outfin = an + sbias; out-proj (plain fp8) accumulates into
            # yT as soon as each head pair completes.
            outfin_sb = bigp.tile([64, H, TQ], F8)
            with (
                tc.tile_pool(name="pmlp", bufs=1, space="PSUM") as pmlp,
                tc.tile_pool(name="hpool", bufs=2) as hpool,
            ):
                yT = pmlp.tile([128, 2, TQ], F32, tag="yT", bufs=1)
                wv = yT[:, 0, 0:64].bitcast(BF16)
                for _ in range(30):
                    nc.tensor.transpose(wv, ident, ident)

                # flat (head, kc-pair) pipeline with one-iteration h1
                # lookahead: the next h1 pair issues BEFORE h2 of the
                # current one, so the in-order PE queue never head-blocks
                # on a gelu that is still streaming.
                items = [(m, hh, kcp) for m in range(4) for hh in range(2)
                         for kcp in range(2)]

                def issue_h1(i):
                    m, hh, kcp = items[i]
                    h1 = pmlp.tile([128, 2, TQ], F32, tag="h1", bufs=2)
                    for kk in range(2):
                        kc = 2 * kcp + kk
                        nc.tensor.matmul(
                            h1[:, kk, :],
                            spw1_sb[:, kc * 128 : (kc + 1) * 128],
                            dnp_sb[:, m, hh, :],
                            start=True,
                            stop=True,
                        )
                    return h1

                h1_next = issue_h1(0)
                sb_t = None
                pend_out = []
                for i, (m, hh, kcp) in enumerate(items):
                    h = 2 * m + hh
                    if kcp == 0:
                        sb_t = pmlp.tile([64, TQ], F32, tag="sb", bufs=2)
                    h1 = h1_next
                    if i + 1 < len(items):
                        h1_next = issue_h1(i + 1)
                    while pend_out:
                        ph = pend_out.pop()
                        for ec in range(2):
                            nc.tensor.matmul(
                                yT[:, ec, :],
                                wout_sb[:, ph, ec * 128 : (ec + 1) * 128],
                                outfin_sb[:, ph, :],
                                start=(ph == 0),
                                stop=(ph == H - 1),
                            )
                    hsb = hpool.tile([128, 2, TQ], F8, tag="hsb", bufs=3)
                    if has_spb1:
                        for kk in range(2):
                            kc = 2 * kcp + kk
                            nc.scalar.activation(
                                out=hsb[:, kk, :],
                                in_=h1[:, kk, :],
                                func=AF.Gelu,
                                bias=spb1_sb[:, kc : kc + 1],
                            )
                    else:
                        nc.scalar.activation(out=hsb, in_=h1, func=AF.Gelu)
                    nc.tensor.matmul(
                        sb_t,
                        spw2_sb[:, 2 * kcp : 2 * kcp + 2, :],
                        hsb,
                        start=(kcp == 0),
                        stop=(kcp == 1 and not has_spb2),
                        perf_mode=DR,
                    )
                    if kcp == 1:
                        if has_spb2:
                            nc.tensor.matmul(
                                sb_t,
                                spb2_sb,
                                ones_tq,
                                start=False,
                                stop=True,
                                skip_group_check=True,
                            )
                        nc.vector.tensor_tensor(
                            out=outfin_sb[:, h, :],
                            in0=an_sb[:, m, hh, :],
                            in1=sb_t,
                            op=OP.add,
                        )
                        pend_out.append(h)
                    if i == 7:
                        # pass-3 normalization arithmetic lands here so
                        # heads 0..3's outfin adds aren't queued behind
                        # it on the DVE (heads 6/7 need it from i=12).
                        norm_rest(prev3[0], prev3_rrow)
                for ph in pend_out:
                    for ec in range(2):
                        nc.tensor.matmul(
                            yT[:, ec, :],
                            wout_sb[:, ph, ec * 128 : (ec + 1) * 128],
                            outfin_sb[:, ph, :],
                            start=(ph == 0),
                            stop=(ph == H - 1),
                        )

                # ---- final gelu (fused 2^-11 descale) + residual ----
                for ec in range(2):
                    ysb = workp.tile([128, TQ], F32, tag="ysb")
                    nc.scalar.activation(
                        out=ysb,
                        in_=yT[:, ec, :],
                        func=AF.Gelu,
                        bias=outb_sb[:, ec : ec + 1],
                        scale=YT_INV,
                    )
                    res = workp.tile([128, TQ], F32, tag="res")
                    nc.vector.tensor_tensor(
                        out=res, in0=ysb, in1=featt_sb[:, ec, :], op=OP.add
                    )
                    nc.sync.dma_start(out=out_d[:, ec, :], in_=res)
            rsp_cm.__exit__(None, None, None)

    nc.compile()
    return nc


def prepare_maps(inputs):
    xyzs = np.asarray(inputs["xyzs"], np.float32)
    features = np.asarray(inputs["features"], np.float32)
    ln_g = np.asarray(inputs["ln_g"], np.float32)
    ln_b = np.asarray(inputs["ln_b"], np.float32)
    w_qkv = np.asarray(inputs["w_qkv"], np.float32)
    sp_w1 = np.asarray(inputs["sp_w1"], np.float32)
    sp_b1 = np.asarray(inputs["sp_b1"], np.float32)
    sp_w2 = np.asarray(inputs["sp_w2"], np.float32)
    sp_b2 = np.asarray(inputs["sp_b2"], np.float32)
    out_w = np.asarray(inputs["out_w"], np.float32)
    out_b = np.asarray(inputs["out_b"], np.float32)

    scale = DH ** -0.5
    wqkv_f = w_qkv * ln_g[:, None]
    wqkv_f[:, :INNER] = wqkv_f[:, :INNER] * scale
    bqkv = (ln_b @ w_qkv).astype(np.float32)
    bqkv[:INNER] *= scale

    has_bqkv = bool(np.any(bqkv != 0.0))
    has_spb1 = bool(np.any(sp_b1 != 0.0))
    has_spb2 = bool(np.any(sp_b2 != 0.0))

    cf32 = np.zeros((128, 16), np.float32)
    for oc in range(4):
        cf32[:, oc] = bqkv[oc * 128 : (oc + 1) * 128]
        cf32[:, 4 + oc] = bqkv[INNER + oc * 128 : INNER + (oc + 1) * 128]
    for kc in range(4):
        cf32[:, 8 + kc] = sp_b1[kc * 128 : (kc + 1) * 128]
    cf32[:, 12] = out_b[:128]
    cf32[:, 13] = out_b[128:]

    cbf = np.zeros((1, TQ + INNER + DH), np.float32)
    cbf[0, 0:TQ] = 1.0
    cbf[0, TQ : TQ + INNER] = bqkv[2 * INNER :] * WQ_S
    cbf[0, TQ + INNER :] = sp_b2 * W2_S

    # wout as [64, H, 256]: row (d, h) = out_w[h*64+d, :]
    wout64 = np.ascontiguousarray(out_w.reshape(H, 64, DIM).transpose(1, 0, 2))

    wqkv8 = (wqkv_f * WQ_S).reshape(2, 128, 3 * INNER).transpose(1, 0, 2)
    spw232 = (sp_w2 * W2_S).reshape(4, 128, DH).transpose(1, 0, 2)
    shared = {
        "wqkv": np.ascontiguousarray(wqkv8).astype(E4),
        "cf32": cf32,
        "cbf": cbf.astype(BF),
        "spw1": np.ascontiguousarray(
            np.concatenate([sp_w1, np.zeros((1, 2 * DIM), np.float32)])
        ).astype(BF),
        "spw2": np.ascontiguousarray(spw232).astype(E4),
        "wout": (wout64 * WO_S).astype(E4),
    }

    in_maps = []
    for core in range(N_CORES):
        bi, quarter = core // 4, core % 4
        qs = quarter * TQ
        x_b = features[bi].reshape(M, DIM)
        xyz_b = xyzs[bi].reshape(M, 3)
        x_perm = np.roll(x_b, -qs, axis=0)
        xyz_perm = np.roll(xyz_b, -qs, axis=0)
        xyza = np.concatenate(
            [xyz_perm / OF_S, np.full((M, 1), 1.0 / OF_S, np.float32)], axis=1
        ).astype(np.float32)
        m = dict(shared)
        m["x"] = np.ascontiguousarray(
            x_perm.reshape(NT, 128, DIM).transpose(1, 0, 2)
        ).astype(BF)
        m["xyzv"] = np.ascontiguousarray(
            xyza.reshape(NT, 128, 4).transpose(1, 0, 2)
        ).astype(E4)
        m["xyzt"] = np.ascontiguousarray(
            np.concatenate([xyz_perm[:TQ].T, np.zeros((1, TQ), np.float32)])
        ).astype(BF)
        m["featt"] = np.ascontiguousarray(
            x_perm[:TQ].T.reshape(2, 128, TQ).transpose(1, 0, 2)
        )
        in_maps.append(m)
    return in_maps, (has_bqkv, has_spb1, has_spb2)


def assemble(results, l=16, n=128):
    out = np.zeros((2, M, DIM), np.float32)
    for core in range(N_CORES):
        bi, quarter = core // 4, core % 4
        qs = quarter * TQ
        o = results[core]["out"]  # [128, 2, TQ]
        out[bi, qs : qs + TQ, :] = (
            o.transpose(1, 0, 2).reshape(DIM, TQ).T
        )
    return out.reshape(2, l, n, DIM)


def kernel(**inputs):
    in_maps, flags = prepare_maps(inputs)
    nc = build_program(*flags)
    results = run_bass_kernel_spmd(nc, in_maps, list(range(N_CORES))).results
    return assemble(results)


if __name__ == "__main__":
    pass
